# revision 7
# baseline (speedup 1.0000x reference)
"""Trainium2 Bass kernel for int4-grouped-quantized linear (GPTQ-style).

out[8192, 11008] = x[8192, 4096] @ dequant(qweight, qzeros, scales)

Sharding: column-parallel over out_features N across 8 NeuronCores.

Device-side structure per core:
  - Dequantize the W shard [4096, 1376] on-chip: qweight is viewed as bytes
    (host-side reinterpret), so each nibble extraction is a single int ALU op
    and fuses with the scale multiply via scalar_tensor_tensor:
        w*s = (qw_u8 & 0xF) * s_bc   /   (qw_u8 >> 4) * s_bc
    then one tensor_tensor subtract of the precomputed broadcast zs = z*s.
    This orders W's columns [all lo-nibbles | all hi-nibbles]; the host
    un-permutes output columns (pure reshape/transpose).
  - Dense fp16 matmuls on the PE with fp32 PSUM accumulation; x is
    pre-transposed/tiled on the host (layout only) so the stationary
    [128 k, 128 t] tiles stream straight from DRAM, no on-device transposes.
"""

import sys

sys.path.insert(0, "/opt/trn_rl_repo")

from contextlib import ExitStack

import numpy as np

import concourse.bass as bass
from concourse import bacc
import concourse.tile as tile
from concourse import mybir
from concourse.bass_utils import run_bass_kernel_spmd

AOT = mybir.AluOpType
F16, I32, U8 = mybir.dt.float16, mybir.dt.int32, mybir.dt.uint8
F32 = mybir.dt.float32

T, K, N = 8192, 4096, 11008
NCORES = 8
NS = N // NCORES  # 1376 out cols per core
CS = NS // 8  # 172 packed int32 cols per core
CB = CS * 4  # 688 packed bytes per core (= NS/2)
G = 32  # quant groups (group size 128 == one k-block)
KB = K // 128  # 32 k-blocks
QCH = 8  # k-blocks per qweight load chunk
TC = 512  # t columns per x.T chunk
NCH = T // TC  # 16 chunks
TBLK = TC // 128  # 4 output row-blocks per chunk
SEGS = [(0, 512), (512, 512), (1024, 352)]  # N segments (PSUM bank sized)


def _body(ctx, tc, xtd, qwd, qzd, scd, outd, comb):
    nc = tc.nc
    cpool = ctx.enter_context(tc.tile_pool(name="const", bufs=1))
    qpool = ctx.enter_context(tc.tile_pool(name="qwp", bufs=2))
    stpool = ctx.enter_context(tc.tile_pool(name="stage", bufs=2))
    wpool = ctx.enter_context(tc.tile_pool(name="w", bufs=KB))
    bcpool = ctx.enter_context(tc.tile_pool(name="bc", bufs=2))
    xpool = ctx.enter_context(tc.tile_pool(name="x", bufs=2))
    pspool = ctx.enter_context(tc.tile_pool(name="ps", bufs=2, space="PSUM"))
    opool = ctx.enter_context(tc.tile_pool(name="o", bufs=3))

    # ---- zero-points: unpack qz bytes -> z [G, NS] f16 (permuted layout),
    #      zs = z*s, park [s | zs] in DRAM for per-block partition broadcast ----
    qz_t = cpool.tile([G, CB], U8)
    nc.gpsimd.dma_start(qz_t[:], qzd)
    s_t = cpool.tile([G, NS], F16)
    nc.gpsimd.dma_start(s_t[:], scd)
    z_t = cpool.tile([G, NS], U8)
    nc.vector.tensor_scalar(z_t[:, :CB], qz_t[:], 0xF, None, AOT.bitwise_and)
    nc.vector.tensor_scalar(z_t[:, CB:], qz_t[:], 4, None, AOT.logical_shift_right)
    zs_t = cpool.tile([G, NS], F16)
    nc.vector.tensor_tensor(zs_t[:], z_t[:], s_t[:], AOT.mult)
    nc.gpsimd.dma_start(comb[:, :NS], s_t[:])
    nc.gpsimd.dma_start(comb[:, NS:], zs_t[:])

    # ---- dequantize W: w = w4*s - z*s, one k-block (= one quant group) each ----
    w_tiles = []
    for q in range(KB // QCH):
        qw_t = qpool.tile([128, QCH * CB], U8)
        nc.gpsimd.dma_start(
            qw_t[:].rearrange("p (b c) -> p b c", b=QCH),
            qwd[q * QCH * 128 : (q + 1) * QCH * 128, :].rearrange(
                "(b p) c -> p b c", p=128
            ),
        )
        for i in range(QCH):
            b = q * QCH + i
            qw_b = qw_t[:, i * CB : (i + 1) * CB]
            bc = bcpool.tile([128, 2 * NS], F16)
            nc.gpsimd.dma_start(bc[:], comb[b : b + 1, :].partition_broadcast(128))
            wst = stpool.tile([128, NS], U8)
            nc.vector.tensor_scalar(wst[:, :CB], qw_b, 0xF, None, AOT.bitwise_and)
            nc.vector.tensor_scalar(
                wst[:, CB:], qw_b, 4, None, AOT.logical_shift_right
            )
            w_t = wpool.tile([128, NS], F16)
            nc.vector.tensor_tensor(w_t[:], wst[:], bc[:, :NS], AOT.mult)
            nc.vector.tensor_tensor(w_t[:], w_t[:], bc[:, NS:], AOT.subtract)
            w_tiles.append(w_t)

    # ---- matmul: stream pre-transposed x chunks, accumulate over K ----
    for c in range(NCH):
        xt_t = xpool.tile([128, KB * TC], F16, tag="xt")
        nc.gpsimd.dma_start(xt_t[:], xtd[c * 128 : (c + 1) * 128, :])
        for tau in range(TBLK):
            ps = pspool.tile([128, NS], F32)
            for b in range(KB):
                lhs = xt_t[:, b * TC + tau * 128 : b * TC + (tau + 1) * 128]
                for off, sz in SEGS:
                    nc.tensor.matmul(
                        ps[:, off : off + sz],
                        lhs,
                        w_tiles[b][:, off : off + sz],
                        start=(b == 0),
                        stop=(b == KB - 1),
                    )
            ob = opool.tile([128, NS], F16)
            nc.vector.tensor_copy(ob[:], ps[:])
            r0 = c * TC + tau * 128
            nc.gpsimd.dma_start(outd[r0 : r0 + 128, :], ob[:])


def build_kernel():
    nc = bacc.Bacc("TRN2", target_bir_lowering=False, debug=False)
    xtd = nc.dram_tensor("xt", [NCH * 128, KB * TC], F16, kind="ExternalInput").ap()
    qwd = nc.dram_tensor("qw", [K, CB], U8, kind="ExternalInput").ap()
    qzd = nc.dram_tensor("qz", [G, CB], U8, kind="ExternalInput").ap()
    scd = nc.dram_tensor("sc", [G, NS], F16, kind="ExternalInput").ap()
    outd = nc.dram_tensor("out", [T, NS], F16, kind="ExternalOutput").ap()
    comb = nc.dram_tensor("comb_scratch", [G, 2 * NS], F16, kind="Internal").ap()
    with tile.TileContext(nc) as tc, ExitStack() as ctx:
        _body(ctx, tc, xtd, qwd, qzd, scd, outd, comb)
    nc.compile()
    return nc


_NC = None


def _get_nc():
    global _NC
    if _NC is None:
        _NC = build_kernel()
    return _NC


def _tile_xt(x):
    # x [T, K] -> xt [NCH*128, KB*TC] where
    # xt[c*128 + p, b*TC + t] = x[c*TC + t, b*128 + p]
    xt = np.ascontiguousarray(
        x.reshape(NCH, TC, KB, 128).transpose(0, 3, 2, 1)
    ).reshape(NCH * 128, KB * TC)
    return xt


def _perm_cols(a):
    # reference col n = c*8 + j -> device col: lo nibbles (j=2k) first, hi after
    # a [..., NS] -> [..., NS] with device order [c*4+k | CB + c*4+k]
    lead = a.shape[:-1]
    return np.ascontiguousarray(
        a.reshape(*lead, CS, 4, 2).transpose(*range(len(lead)), -1, -3, -2)
    ).reshape(*lead, NS)


def _unperm_out(o):
    # o [T, NS] device order -> reference column order
    return o.reshape(T, 2, CS, 4).transpose(0, 2, 3, 1).reshape(T, NS)


def make_in_maps(x, qweight, qzeros, scales):
    x = np.asarray(x, dtype=np.float16)
    qweight = np.asarray(qweight, dtype=np.int32)
    qzeros = np.asarray(qzeros, dtype=np.int32)
    scales = np.asarray(scales, dtype=np.float16)
    xt = _tile_xt(x)
    in_maps = []
    for c in range(NCORES):
        qw = np.ascontiguousarray(qweight[:, c * CS : (c + 1) * CS])
        qz = np.ascontiguousarray(qzeros[:, c * CS : (c + 1) * CS])
        sc = scales[:, c * NS : (c + 1) * NS]
        in_maps.append(
            {
                "xt": xt,
                "qw": qw.view(np.uint8).reshape(K, CB),
                "qz": qz.view(np.uint8).reshape(G, CB),
                "sc": _perm_cols(sc),
            }
        )
    return in_maps


def run(in_maps, **kwargs):
    return run_bass_kernel_spmd(
        _get_nc(), in_maps, core_ids=list(range(NCORES)), **kwargs
    )


def assemble(res):
    outs = [_unperm_out(res.results[c]["out"]) for c in range(NCORES)]
    return np.concatenate(outs, axis=1)


def kernel(x, qweight, qzeros, scales):
    res = run(make_in_maps(x, qweight, qzeros, scales))
    return assemble(res)
